# revision 16
# baseline (speedup 1.0000x reference)
"""Trainium2 Bass kernel: FiLM-conditioned 1x1-conv self-attention block.

Sharding: 8 cores = 2 batches x 4 heads. Each core computes one (batch, head)
pair end-to-end, producing a partial output projection [C, N]; the host sums
the 4 head partials per batch (b_out is added on the h==0 cores only).

Math notes:
  - FiLM is applied directly to x during the fp32->bf16 conversion pass:
    x~ = x*(1+scale) + shift as one tensor_scalar/activation op with
    per-partition scale+bias APs -- the conversion pass the kernel needed
    anyway, so FiLM is free and no weight folding / rank-1 bias tricks are
    needed downstream.
  - silu(t) = t * 1/(1+exp(-t)) via the EXP table only (identity/copy live
    in the same ACT table set, so the table never reloads mid-kernel).
  - Attention in transposed layout S^T[j,i] = sum_d k[d,j] q[d,i]; softmax
    scale 1/sqrt(d) folded into the exp activation's free scale. No
    max-subtraction (scores are O(5), exp is safe in fp32).
  - U = [V; 1]^T @ P^T accumulated over j gives the unnormalized output and
    the softmax denominator (row 32) in one matmul chain; normalization is
    deferred past the output projection (columnwise 1/den commutes with Wo),
    and the bo row of the augmented projection weight rides the denominator.

Performance structure:
  - scores run as fp8e4m3 DoubleRow matmuls (2x PE throughput): q/k are
    written fp8 with d split into two 16-lane halves along the free axis
    (layout [16p, 2, n]); reps for the two PE row-groups live at partition
    bands {0:16, 64:80}. All partition slices start at multiples of 32.
  - scores for 4 j-tiles land in two [128, 2*NT] PSUM tiles -> wide exps.
  - S and U PSUM double-buffered; denominator kept in f32; the [1, NT]
    reciprocal runs on idle DVE mid-block; proj at block end.
  - epilogue: last i-tile normalize/project/DMA in 128-col chunks pipelined
    across PE/DVE/Pool/DMA.
"""

import functools
import sys

import numpy as np

if "/opt/trn_rl_repo" not in sys.path:
    sys.path.insert(0, "/opt/trn_rl_repo")

HEADS = 4
D = 32              # dim head
C = 256             # channels
TD = 512            # time embedding dim
SCALE = D ** -0.5
N_FULL = 4096       # 64*64 spatial positions
NT = 512            # query (i) tile
JT = 128            # key (j) tile
N_CORES = 8


def _build_program(n_pos=N_FULL):
    import concourse.bass as bass
    import concourse.tile as tile
    from concourse import bacc, mybir
    from concourse.masks import make_identity

    f32 = mybir.dt.float32
    bf16 = mybir.dt.bfloat16
    fp8 = mybir.dt.float8e4
    DR = mybir.MatmulPerfMode.DoubleRow
    AF = mybir.ActivationFunctionType
    ALU = mybir.AluOpType

    nc = bacc.Bacc("TRN2", debug=False)

    xb = nc.dram_tensor("xb", [C, n_pos], f32, kind="ExternalInput").ap()
    te = nc.dram_tensor("te", [TD], f32, kind="ExternalInput").ap()
    w_mlp = nc.dram_tensor("w_mlp", [TD, TD], f32, kind="ExternalInput").ap()
    b_mlp = nc.dram_tensor("b_mlp", [TD], f32, kind="ExternalInput").ap()
    wqkv = nc.dram_tensor("wqkv", [3 * D, C], f32, kind="ExternalInput").ap()
    wo = nc.dram_tensor("wo", [C, D], f32, kind="ExternalInput").ap()
    bo = nc.dram_tensor("bo", [C], f32, kind="ExternalInput").ap()
    out = nc.dram_tensor("out", [C, n_pos], f32, kind="ExternalOutput").ap()

    n_itiles = n_pos // NT
    n_jtiles = n_pos // JT
    n_quads = n_jtiles // 4
    XCHUNK = min(1024, n_pos)
    n_xchunks = n_pos // XCHUNK

    with tile.TileContext(nc) as tc:
        with tc.tile_pool(name="const", bufs=1) as const, \
             tc.tile_pool(name="xio", bufs=4) as xio:
            ident = const.tile([128, 128], f32, name="ident")
            make_identity(nc, ident)
            ident_bf = const.tile([128, 128], bf16, name="ident_bf")
            make_identity(nc, ident_bf)

            # FiLM'd x chunks (bf16) + fp32 staging
            x_sb = [[const.tile([128, XCHUNK], bf16, name=f"x_sb{cc}_{k}")
                     for k in range(n_xchunks)] for cc in range(2)]
            # fp8 q/k for DoubleRow score matmuls: free layout (h, pos) with
            # h = the d-halves (d = 16h + t); the two PE row-group reps live
            # at partition bands {0:16} and {64:80}
            q4t = [const.tile([128, 2 * NT], fp8, name=f"q4_{it}")
                   for it in range(n_itiles)]
            k4 = const.tile([128, 2 * n_pos], fp8, name="k4")
            k4h = k4.rearrange("p (h j) -> p h j", h=2)
            # V with a ones row appended (row D): the PE transpose emits
            # [128, D+1] V^T tiles with the ones column built in. Row D+1 is
            # 4B-alignment padding, never read.
            v_sb = const.tile([D + 2, n_pos], bf16, name="v_sb")
            VTS = D + 2  # vt block stride
            vt_all = const.tile([128, n_jtiles * VTS], bf16, name="vt_all")

            # FiLM results: sc1 = 1+scale (cols by c-chunk); shift stays in
            # tfull cols 2,3
            tfull = const.tile([128, 4], f32, name="tfull")
            sc1 = const.tile([128, 2], f32, name="sc1")
            # qkv weights, transposed, 16-col halves duplicated across the
            # 128 stationary columns (only bands {0:16, 64:80} of the
            # projection output are consumed)
            Wq4 = [[const.tile([128, 128], bf16, name=f"Wq4_{hh}_{cc}")
                    for cc in range(2)] for hh in range(2)]
            Wk4 = [[const.tile([128, 128], bf16, name=f"Wk4_{hh}_{cc}")
                    for cc in range(2)] for hh in range(2)]
            vT_w = [const.tile([128, D], bf16, name=f"vT_w{cc}")
                    for cc in range(2)]
            # proj weights with a bo row appended: row D carries b_out so the
            # deferred normalization turns bo*denom into bo exactly
            woT_aug = [const.tile([D + 1, 128], bf16, name=f"woT{hh}")
                       for hh in range(2)]

            # x fp32 staging chunks (DMA'd up front, converted lazily)
            x_stage = [[xio.tile([128, XCHUNK], f32, name=f"x_st_{cc}_{k}",
                                 tag=f"x_st_{cc}_{k}", bufs=1)
                        for k in range(n_xchunks)] for cc in range(2)]

            # -------- DMAs: everything in flight immediately --------
            te_t = const.tile([128, 4], f32, name="te_t")
            nc.sync.dma_start(out=te_t, in_=te.rearrange("(f p) -> p f", p=128))
            # w_mlp first: it gates the FiLM chain (which now also gates the
            # x converts) and shares HBM bandwidth with everything earlier.
            wm_slab = []
            for ot in range(4):
                t_ = xio.tile([128, TD], f32, name=f"wm_slab_{ot}",
                              tag=f"wm_slab_{ot}", bufs=1)
                nc.sync.dma_start(out=t_, in_=w_mlp[ot * 128:(ot + 1) * 128, :])
                wm_slab.append(t_)
            wqkv_nat = const.tile([3 * D, C], f32, name="wqkv_nat")
            nc.sync.dma_start(out=wqkv_nat, in_=wqkv)
            bm_t = const.tile([128, 4], f32, name="bm_t")
            nc.sync.dma_start(out=bm_t, in_=b_mlp.rearrange("(f p) -> p f", p=128))
            n_xsync = min(2, n_xchunks)
            for k in range(n_xsync):
                for cc in range(2):
                    nc.sync.dma_start(
                        out=x_stage[cc][k],
                        in_=xb[128 * cc:128 * (cc + 1),
                               k * XCHUNK:(k + 1) * XCHUNK])
            wo_both = const.tile([128, 2 * D], f32, name="wo_both")
            nc.sync.dma_start(
                out=wo_both.rearrange("p (h d) -> p h d", d=D),
                in_=wo.rearrange("(h p) d -> p h d", p=128))
            bo_dma = const.tile([128, 2], f32, name="bo_dma")
            nc.sync.dma_start(out=bo_dma, in_=bo.rearrange("(f p) -> p f", p=128))
            for k in range(n_xsync, n_xchunks):
                for cc in range(2):
                    nc.sync.dma_start(
                        out=x_stage[cc][k],
                        in_=xb[128 * cc:128 * (cc + 1),
                               k * XCHUNK:(k + 1) * XCHUNK])

            # PE pipeline warm-up while the DMAs land: the first real
            # matmuls then run at full clock instead of the cold pstate
            with tc.tile_pool(name="warm_ps", bufs=2, space="PSUM") as warm_ps:
                for wi in range(12):
                    wt = warm_ps.tile([128, 128], bf16, tag="w",
                                      name=f"warm_{wi}")
                    nc.tensor.transpose(wt, ident_bf, ident_bf)

            # ones + pad rows of V on the otherwise-idle gpsimd engine
            # (partition slices must start at multiples of 32 -> both rows)
            nc.gpsimd.memset(v_sb[D:D + 2, :], 1.0)

            # ---------------- prologue: FiLM MLP + weight prep --------------
            with tc.tile_pool(name="pro_sb", bufs=3) as pro_sb, \
                 tc.tile_pool(name="pro_ps", bufs=2, space="PSUM") as pro_ps:

                # silu(te) via the EXP table only: s = te / (1 + exp(-te))
                emt = pro_sb.tile([128, 4], f32, tag="emt")
                nc.scalar.activation(emt, te_t, AF.Exp, scale=-1.0)
                nc.vector.tensor_scalar_add(emt, emt, 1.0)
                rec = pro_sb.tile([128, 4], f32, tag="rec")
                nc.vector.reciprocal(rec, emt)
                s_t = const.tile([128, 4], f32, name="s_t")
                nc.vector.tensor_mul(s_t, te_t, rec)
                s_bf = pro_sb.tile([128, 4], bf16, tag="s_bf")
                nc.vector.tensor_copy(s_bf, s_t)

                # W_mlp^T tiles via bf16 PE transpose
                wm_bf = []
                for ot in range(4):
                    wb = pro_sb.tile([128, TD], bf16, tag=f"wm_bf_{ot}",
                                     name=f"wm_bf_{ot}", bufs=1)
                    nc.vector.tensor_copy(wb, wm_slab[ot])
                    wm_bf.append(wb)
                wmT = [[None] * 4 for _ in range(4)]
                for cc4 in range(4):
                    for ot in range(4):
                        ps_t = pro_ps.tile([128, 128], bf16, tag="tp", bufs=3)
                        nc.tensor.transpose(
                            ps_t,
                            wm_bf[ot][:, cc4 * 128:(cc4 + 1) * 128], ident_bf)
                        wmT_t = pro_sb.tile([128, 128], bf16,
                                            tag=f"wmT_{cc4}_{ot}",
                                            name=f"wmT_{cc4}_{ot}", bufs=1)
                        nc.vector.tensor_copy(wmT_t, ps_t)
                        wmT[cc4][ot] = wmT_t

                # t = W_mlp @ silu(te) + b_mlp
                for ot in range(4):
                    t_ps = pro_ps.tile([128, 1], f32, tag="t_ps")
                    for cc4 in range(4):
                        nc.tensor.matmul(
                            t_ps, wmT[cc4][ot], s_bf[:, cc4:cc4 + 1],
                            start=(cc4 == 0), stop=(cc4 == 3),
                        )
                    nc.vector.tensor_add(
                        tfull[:, ot:ot + 1], t_ps, bm_t[:, ot:ot + 1]
                    )
                # 1+scale for the two c-chunks; shift = tfull cols 2,3
                nc.vector.tensor_scalar_add(sc1, tfull[:, 0:2], 1.0)
                shf = tfull[:, 2:4]

                # W_qkv head slices -> bf16 -> transpose; q/k 16-col halves
                # duplicated across the stationary tiles, v plain
                wqkv_bf = pro_sb.tile([3 * D, C], bf16, tag="wqkv_bf")
                nc.vector.tensor_copy(wqkv_bf, wqkv_nat)
                w_bf = {"q": wqkv_bf[0:D, :], "k": wqkv_bf[D:2 * D, :],
                        "v": wqkv_bf[2 * D:3 * D, :]}
                w_base = {"q": 0, "k": D, "v": 2 * D}
                for name, Wdst in (("k", Wk4), ("q", Wq4), ("v", None)):
                    b0 = w_base[name]
                    for cc in range(2):
                        ps_t = pro_ps.tile([128, D], bf16, tag="tp", bufs=3)
                        nc.tensor.transpose(
                            ps_t, w_bf[name][:, 128 * cc:128 * (cc + 1)],
                            ident_bf[b0:b0 + D, b0:b0 + D],
                        )
                        if name == "v":
                            nc.vector.tensor_copy(vT_w[cc], ps_t)
                        else:
                            for hh in range(2):
                                w = Wdst[hh][cc]
                                nc.vector.tensor_copy(
                                    w[:, 0:16], ps_t[:, 16 * hh:16 * hh + 16])
                                nc.vector.tensor_copy(w[:, 16:32], w[:, 0:16])
                                nc.vector.tensor_copy(w[:, 32:64], w[:, 0:32])
                                nc.vector.tensor_copy(w[:, 64:128], w[:, 0:64])

                # w_out^T halves + bo aug row
                wo_bf = pro_sb.tile([128, 2 * D], bf16, tag="wo_bf")
                nc.vector.tensor_copy(wo_bf, wo_both)
                for hh in range(2):
                    ps_t = pro_ps.tile([D, 128], bf16, tag="tp", bufs=3)
                    nc.tensor.transpose(ps_t,
                                        wo_bf[:, D * hh:D * (hh + 1)], ident_bf)
                    nc.vector.tensor_copy(woT_aug[hh][0:D, :], ps_t)
                    bo_bf = pro_sb.tile([128, 1], bf16,
                                        tag=f"bo_bf{hh}", bufs=1)
                    nc.vector.tensor_copy(bo_bf, bo_dma[:, hh:hh + 1])
                    ps_r = pro_ps.tile([D + 1, 128], bf16, tag="tp", bufs=3)
                    nc.tensor.transpose(ps_r[D:D + 1, :], bo_bf, ident_bf,
                                        tile_position=(0, 32))
                    nc.vector.tensor_copy(woT_aug[hh][D:D + 1, :],
                                          ps_r[D:D + 1, :])

            # ---------------- attention + output projection ----------------
            with tc.tile_pool(name="sc_ps", bufs=3, space="PSUM") as sc_ps, \
                 tc.tile_pool(name="u_ps", bufs=2, space="PSUM") as u_ps, \
                 tc.tile_pool(name="pt_sb", bufs=6) as pt_sb, \
                 tc.tile_pool(name="o_sb", bufs=3) as o_sb:

                converted = [False] * n_xchunks

                def film_act(k):
                    # FiLM'd conversion on the scalar engine:
                    # x~ = Identity(x * (1+scale) + shift), fp32 -> bf16
                    for cc in range(2):
                        nc.scalar.activation(
                            x_sb[cc][k], x_stage[cc][k], AF.Identity,
                            bias=shf[:, cc:cc + 1], scale=sc1[:, cc:cc + 1])

                def ensure_chunk(k):
                    if converted[k]:
                        return
                    converted[k] = True
                    # FiLM'd conversion on DVE
                    for cc in range(2):
                        nc.vector.tensor_scalar(
                            x_sb[cc][k], x_stage[cc][k],
                            sc1[:, cc:cc + 1], shf[:, cc:cc + 1],
                            ALU.mult, ALU.add)

                # first chunks on the (pre-loop idle) scalar engine
                n_pro = n_xsync
                for k in range(n_pro):
                    converted[k] = True
                    film_act(k)

                def emit_qkv(nt):
                    # qkv projection for one n-tile; k first (it gates the
                    # i-tile-0 score matmuls), then q and v. Both d-halves of
                    # k (and of q) land in one [128, 2*NT] PSUM tile; one
                    # wide fp8 copy each.
                    ensure_chunk(nt * NT // XCHUNK)
                    ensure_chunk(((nt + 1) * NT - 1) // XCHUNK)
                    sl = slice(nt * NT, (nt + 1) * NT)
                    kx = nt * NT // XCHUNK
                    lo = nt * NT - kx * XCHUNK
                    xch = [x_sb[cc][kx][:, lo:lo + NT] for cc in range(2)]
                    ps_k = sc_ps.tile([128, 2 * NT], f32, tag="sc",
                                      name=f"psk_{nt}")
                    for hh in range(2):
                        for cc in range(2):
                            nc.tensor.matmul(
                                ps_k[:, hh * NT:(hh + 1) * NT],
                                Wk4[hh][cc], xch[cc],
                                start=(cc == 0), stop=(cc == 1),
                                skip_group_check=True)
                    nc.vector.tensor_copy(
                        k4h[:, :, sl],
                        ps_k.rearrange("p (h i) -> p h i", h=2))
                    # (strided 1B writes: dst pairs interleaved)
                    ps_q = sc_ps.tile([128, 2 * NT], f32, tag="sc",
                                      name=f"psq_{nt}")
                    for hh in range(2):
                        for cc in range(2):
                            nc.tensor.matmul(
                                ps_q[:, hh * NT:(hh + 1) * NT],
                                Wq4[hh][cc], xch[cc],
                                start=(cc == 0), stop=(cc == 1),
                                skip_group_check=True)
                    nc.vector.tensor_copy(
                        q4t[nt].rearrange("p (i h) -> p h i", h=2),
                        ps_q.rearrange("p (h i) -> p h i", h=2))
                    ps_v = sc_ps.tile([D, NT], f32, tag="sc",
                                      name=f"psv_{nt}")
                    for cc in range(2):
                        nc.tensor.matmul(ps_v, vT_w[cc], xch[cc],
                                         start=(cc == 0), stop=(cc == 1))
                    nc.vector.tensor_copy(v_sb[0:D, sl], ps_v)

                def emit_vt(nt):
                    # V^T for the 4 j-tiles of this n-tile; AFTER the score
                    # matmuls of the group so the in-order PE queue never
                    # delays scores on v.
                    ps_vt = sc_ps.tile([128, 4 * VTS], bf16, tag="sc",
                                       name=f"psvt_{nt}")
                    for jj in range(4):
                        j = 4 * nt + jj
                        nc.tensor.transpose(
                            ps_vt[:, jj * VTS:(jj + 1) * VTS],
                            v_sb[:, j * JT:(j + 1) * JT],
                            ident_bf[0:VTS, 0:VTS])
                    nc.vector.tensor_copy(
                        vt_all[:, 4 * nt * VTS:(4 * nt + 4) * VTS],
                        ps_vt)

                def score_mm(Sx, off, j, r, qt):
                    rr = 64 * (r % 2)
                    nc.tensor.matmul(
                        Sx[:, off:off + NT],
                        k4h[rr:rr + 16, :, j * JT:(j + 1) * JT],
                        qt[rr:rr + 16, :].rearrange("p (i h) -> p h i", h=2),
                        start=True, stop=True, perf_mode=DR,
                        tile_position=(rr, 0),
                    )

                def emit_scores0(g):
                    # i-tile-0 scores with a consumerless dummy allocation
                    # between S1 and S2: the pool rotation then maps every
                    # reuse onto an early DVE copy instead of an exp
                    # completion, decoupling the qkv build from the exp
                    # cadence.
                    S1 = sc_ps.tile([128, 2 * NT], f32, tag="sc",
                                    name=f"S1_0_{g}")
                    for r, off in ((0, 0), (1, NT)):
                        score_mm(S1, off, 4 * g + r, r, q4t[0])
                    sc_ps.tile([128, 8], f32, tag="sc", name=f"dummy_{g}")
                    S2 = sc_ps.tile([128, 2 * NT], f32, tag="sc",
                                    name=f"S2_0_{g}")
                    for r, off in ((2, 0), (3, NT)):
                        score_mm(S2, off, 4 * g + r, r, q4t[0])
                    PT1 = pt_sb.tile([128, 2 * NT], bf16, tag="pt",
                                     name=f"PT1_0_{g}")
                    nc.scalar.activation(PT1, S1, AF.Exp, scale=SCALE)
                    PT2 = pt_sb.tile([128, 2 * NT], bf16, tag="pt",
                                     name=f"PT2_0_{g}")
                    nc.scalar.activation(PT2, S2, AF.Exp, scale=SCALE)
                    return PT1, PT2

                def emit_scores(it, g):
                    S1 = sc_ps.tile([128, 2 * NT], f32, tag="sc",
                                    name=f"S1_{it}_{g}")
                    S2 = sc_ps.tile([128, 2 * NT], f32, tag="sc",
                                    name=f"S2_{it}_{g}")
                    for r, (Sx, off) in enumerate(
                            ((S1, 0), (S1, NT), (S2, 0), (S2, NT))):
                        score_mm(Sx, off, 4 * g + r, r, q4t[it])
                    PT1 = pt_sb.tile([128, 2 * NT], bf16, tag="pt",
                                     name=f"PT1_{it}_{g}")
                    nc.scalar.activation(PT1, S1, AF.Exp, scale=SCALE)
                    PT2 = pt_sb.tile([128, 2 * NT], bf16, tag="pt",
                                     name=f"PT2_{it}_{g}")
                    nc.scalar.activation(PT2, S2, AF.Exp, scale=SCALE)
                    return PT1, PT2

                def emit_u(U, g, PT1, PT2):
                    st, sp = (g == 0), (g == n_quads - 1)
                    for idx, (PTx, off) in enumerate(
                            ((PT1, 0), (PT1, NT), (PT2, 0), (PT2, NT))):
                        j = 4 * g + idx
                        lo = 0 if idx % 2 == 0 else 64
                        nc.tensor.matmul(
                            U[lo:lo + D + 1, :],
                            vt_all[:, j * VTS:j * VTS + D + 1],
                            PTx[:, off:off + NT],
                            start=(st and idx < 2), stop=(sp and idx >= 2),
                            tile_position=(0, lo),
                            skip_group_check=True,
                        )

                def emit_usum(it, U):
                    # combine the two column-group halves (one PSUM operand
                    # per DVE op); bf16 usum for the proj matmul, f32
                    # denominator row for the reciprocal
                    usum_b = o_sb.tile([D + 1, NT], f32, tag="usum_b",
                                       name=f"usum_b_{it}")
                    nc.vector.tensor_copy(usum_b, U[64:64 + D + 1, :])
                    den = o_sb.tile([1, NT], f32, tag="den", name=f"den_{it}")
                    nc.vector.tensor_add(den, U[D:D + 1, :],
                                         usum_b[D:D + 1, :])
                    usum = o_sb.tile([D + 1, NT], bf16, tag="usum",
                                     name=f"usum_{it}")
                    nc.vector.tensor_add(usum, U[0:D + 1, :], usum_b)
                    return usum, den

                def emit_recip_dve(it, den):
                    # [1, NT] reciprocal: ~2.6us of idle DVE, off the PE path
                    rcp = o_sb.tile([1, NT], f32, tag="rcp", name=f"rcp_{it}")
                    nc.vector.reciprocal(rcp, den)
                    rb = o_sb.tile([128, NT], f32, tag="rb", name=f"rb_{it}")
                    nc.gpsimd.partition_broadcast(rb, rcp)
                    return rb

                def emit_recip_tail(it, den, width, off, sfx):
                    # transposed reciprocal for the tail chunks: PE transpose
                    # -> [128, nq] reciprocal -> transpose back -> broadcast
                    nq = width // 128
                    denT_ps = sc_ps.tile([128, nq], f32, tag="sc",
                                         name=f"denT_{it}{sfx}")
                    for q in range(nq):
                        nc.tensor.transpose(
                            denT_ps[:, q:q + 1],
                            den[:, off + 128 * q:off + 128 * (q + 1)],
                            ident[0:1, 0:1])
                    rcpT = o_sb.tile([128, nq], f32, tag="rcpT",
                                     name=f"rcpT_{it}{sfx}")
                    nc.vector.reciprocal(rcpT, denT_ps)
                    row_ps = sc_ps.tile([1, width], f32, tag="sc",
                                        name=f"rcp_row_{it}{sfx}")
                    for q in range(nq):
                        nc.tensor.transpose(
                            row_ps[:, 128 * q:128 * (q + 1)],
                            rcpT[:, q:q + 1], ident)
                    rcp_row = o_sb.tile([1, width], f32, tag="rcp",
                                        name=f"rcp{it}{sfx}")
                    nc.vector.tensor_copy(rcp_row, row_ps)
                    rb = o_sb.tile([128, width], f32, tag="rb",
                                   name=f"rb_{it}{sfx}")
                    nc.gpsimd.partition_broadcast(rb, rcp_row)
                    return rb

                def emit_proj_mm(pit, usum, width=NT, off=0, sfx=""):
                    ps_os = []
                    for hh in range(2):
                        ps_o = sc_ps.tile([128, width], f32, tag="sc",
                                          name=f"ps_o_{pit}_{hh}{sfx}")
                        nc.tensor.matmul(ps_o, woT_aug[hh],
                                         usum[:, off:off + width],
                                         start=True, stop=True)
                        ps_os.append(ps_o)
                    return ps_os

                def emit_proj_fin(pit, ps_os, rb, width=NT, off=0, sfx=""):
                    psl = slice(pit * NT + off, pit * NT + off + width)
                    for hh in range(2):
                        o_out = o_sb.tile([128, width], f32, tag="o_out",
                                          name=f"o_out_{pit}_{hh}{sfx}")
                        nc.vector.tensor_mul(o_out, ps_os[hh], rb)
                        nc.sync.dma_start(
                            out=out[128 * hh:128 * (hh + 1), psl], in_=o_out
                        )

                # i-tile 0 fused with the qkv/V^T build: quad g only needs
                # k/v/VT of n-tile g (and q of n-tile 0). Software-pipelined:
                # the last U matmuls of i-tile it-1 and its norm/proj are
                # emitted inside i-tile it's block.
                assert n_quads == n_itiles
                U0 = u_ps.tile([97, NT], f32, tag="u", name="U_0")
                emit_qkv(0)
                prev = emit_scores0(0)
                emit_vt(0)
                for g in range(1, n_quads):
                    kx = n_xsync + (g - 2) // 2
                    if g >= 2 and g % 2 == 0 and kx < n_xchunks:
                        # late x chunks: FiLM'd in i-tile 0's ACT idle slots
                        converted[kx] = True
                        film_act(kx)
                    emit_qkv(g)
                    cur = emit_scores0(g)
                    emit_vt(g)
                    emit_u(U0, g - 1, *prev)
                    prev = cur
                carry = (U0, 0, prev)

                for it in range(1, n_itiles):
                    U = u_ps.tile([97, NT], f32, tag="u", name=f"U_{it}")
                    prev = emit_scores(it, 0)
                    Uc, itc, sc_prev = carry
                    emit_u(Uc, n_quads - 1, *sc_prev)
                    usum_c, den_c = emit_usum(itc, Uc)
                    rb_c = None
                    for g in range(1, n_quads):
                        cur = emit_scores(it, g)
                        emit_u(U, g - 1, *prev)
                        prev = cur
                        if g == 3:
                            rb_c = emit_recip_dve(itc, den_c)
                    if rb_c is None:
                        rb_c = emit_recip_dve(itc, den_c)
                    # proj of it-1 at the END of the block: its PSUM slot's
                    # consumer (the normalize multiply) then runs right away,
                    # so the rotation never blocks the next i-tile's scores.
                    pmm = emit_proj_mm(itc, usum_c)
                    emit_proj_fin(itc, pmm, rb_c)
                    carry = (U, it, prev)

                # tail: last i-tile's normalization in 128-col chunks,
                # pipelined across PE/DVE/Pool/DMA
                Uc, itc, sc_prev = carry
                emit_u(Uc, n_quads - 1, *sc_prev)
                usum_t, den_t = emit_usum(itc, Uc)
                CH = 128
                for h2 in range(NT // CH):
                    off = h2 * CH
                    rb_h = emit_recip_tail(itc, den_t, CH, off, f"_t{h2}")
                    pmm = emit_proj_mm(itc, usum_t, width=CH, off=off,
                                       sfx=f"_t{h2}")
                    emit_proj_fin(itc, pmm, rb_h, width=CH, off=off,
                                  sfx=f"_t{h2}")
    nc.compile()
    return nc


@functools.lru_cache(maxsize=2)
def _get_nc(n_pos=N_FULL):
    return _build_program(n_pos)


def _make_in_maps(x, time_emb, w_mlp, b_mlp, w_qkv, w_out, b_out, n_pos=N_FULL):
    x = np.ascontiguousarray(np.asarray(x, dtype=np.float32))
    time_emb = np.ascontiguousarray(np.asarray(time_emb, dtype=np.float32))
    w_mlp = np.ascontiguousarray(np.asarray(w_mlp, dtype=np.float32))
    b_mlp = np.ascontiguousarray(np.asarray(b_mlp, dtype=np.float32))
    w_qkv = np.ascontiguousarray(np.asarray(w_qkv, dtype=np.float32))
    w_out = np.ascontiguousarray(np.asarray(w_out, dtype=np.float32))
    b_out = np.ascontiguousarray(np.asarray(b_out, dtype=np.float32))

    b = x.shape[0]
    hid = HEADS * D
    in_maps = []
    for core in range(N_CORES):
        bb, hh = core // HEADS, core % HEADS
        in_maps.append({
            "xb": np.ascontiguousarray(
                x[bb].reshape(C, -1)[:, :n_pos]),
            "te": time_emb[bb],
            "w_mlp": w_mlp,
            "b_mlp": b_mlp,
            "wqkv": np.ascontiguousarray(np.concatenate([
                w_qkv[D * hh:D * (hh + 1), :],
                w_qkv[hid + D * hh:hid + D * (hh + 1), :],
                w_qkv[2 * hid + D * hh:2 * hid + D * (hh + 1), :]], axis=0)),
            "wo": np.ascontiguousarray(w_out[:, D * hh:D * (hh + 1)]),
            "bo": b_out if hh == 0 else np.zeros_like(b_out),
        })
    return in_maps


def _install_ntff_hook():
    """Register the axon NTFF profile hook (the agent image's antenv lacks
    axon_hooks; replicate trn_boot's ctypes shim so trace=True works)."""
    import types
    import contextlib
    import ctypes

    try:
        from antenv.axon_hooks import get_axon_ntff_profile_hook  # noqa: F401
        return
    except ImportError:
        pass
    so_path = "/opt/axon/libaxon_pjrt.so"
    try:
        lib = ctypes.CDLL(so_path)
    except OSError:
        return
    if not hasattr(lib, "axon_start_nrt_profile"):
        return
    lib.axon_start_nrt_profile.argtypes = [
        ctypes.POINTER(ctypes.c_int64), ctypes.c_size_t]
    lib.axon_start_nrt_profile.restype = ctypes.c_int64
    lib.axon_stop_nrt_profile.argtypes = [ctypes.c_char_p]
    lib.axon_stop_nrt_profile.restype = ctypes.c_int64

    @contextlib.contextmanager
    def _hook(output_dir, device_ids):
        import jax
        jax.devices()
        if device_ids:
            ids = (ctypes.c_int64 * len(device_ids))(*device_ids)
            rc = lib.axon_start_nrt_profile(ids, len(device_ids))
        else:
            rc = lib.axon_start_nrt_profile(None, 0)
        if rc != 0:
            raise RuntimeError(f"axon_start_nrt_profile rc={rc}")
        try:
            yield
        finally:
            n = lib.axon_stop_nrt_profile(str(output_dir).encode())
            print(f"profile: {n} file(s) written to {output_dir}",
                  file=sys.stderr)

    import antenv
    mod = types.ModuleType("antenv.axon_hooks")
    mod.get_axon_ntff_profile_hook = lambda: _hook
    mod.set_axon_ntff_profile_hook = lambda h: None
    sys.modules["antenv.axon_hooks"] = mod
    antenv.axon_hooks = mod


def _run(inputs, trace=False, n_pos=N_FULL):
    from concourse.bass_utils import run_bass_kernel_spmd

    if trace:
        _install_ntff_hook()
    nc = _get_nc(n_pos)
    in_maps = _make_in_maps(**inputs, n_pos=n_pos)
    res = run_bass_kernel_spmd(
        nc, in_maps, core_ids=list(range(N_CORES)), trace=trace
    )
    return res


def _assemble(results, x_shape):
    b, c, h, w = x_shape
    out = np.zeros((b, c, h * w), dtype=np.float32)
    for core in range(N_CORES):
        bb = core // HEADS
        out[bb] += results[core]["out"]
    return out.reshape(b, c, h, w)


def kernel(x, time_emb, w_mlp, b_mlp, w_qkv, w_out, b_out):
    res = _run(dict(
        x=x, time_emb=time_emb, w_mlp=w_mlp, b_mlp=b_mlp,
        w_qkv=w_qkv, w_out=w_out, b_out=b_out,
    ))
    return _assemble(res.results, np.asarray(x).shape)


# revision 17
# speedup vs baseline: 1.0796x; 1.0796x over previous
"""Trainium2 Bass kernel: FiLM-conditioned 1x1-conv self-attention block.

Sharding: 8 cores = 2 batches x 4 heads. Each core computes one (batch, head)
pair end-to-end, producing a partial output projection [C, N]; the host sums
the 4 head partials per batch (b_out is added on the h==0 cores only).

Math notes:
  - FiLM is applied directly to x during the fp32->bf16 conversion pass:
    x~ = x*(1+scale) + shift as one tensor_scalar/activation op with
    per-partition scale+bias APs -- the conversion pass the kernel needed
    anyway, so FiLM is free and no weight folding / rank-1 bias tricks are
    needed downstream.
  - silu(t) = t * 1/(1+exp(-t)) via the EXP table only (identity/copy live
    in the same ACT table set, so the table never reloads mid-kernel).
  - Attention in transposed layout S^T[j,i] = sum_d k[d,j] q[d,i]; softmax
    scale 1/sqrt(d) folded into the exp activation's free scale. No
    max-subtraction (scores are O(5), exp is safe in fp32).
  - U = [V; 1]^T @ P^T accumulated over j gives the unnormalized output and
    the softmax denominator (row 32) in one matmul chain; normalization is
    deferred past the output projection (columnwise 1/den commutes with Wo),
    and the bo row of the augmented projection weight rides the denominator.

Performance structure:
  - scores run as fp8e4m3 DoubleRow matmuls (2x PE throughput): q/k are
    written fp8 with d split into two 16-lane halves along the free axis
    (layout [16p, 2, n]); reps for the two PE row-groups live at partition
    bands {0:16, 64:80}. All partition slices start at multiples of 32.
  - scores for 4 j-tiles land in two [128, 2*NT] PSUM tiles -> wide exps.
  - S and U PSUM double-buffered; denominator kept in f32; the [1, NT]
    reciprocal runs on idle DVE mid-block; proj at block end.
  - epilogue: last i-tile normalize/project/DMA in 128-col chunks pipelined
    across PE/DVE/Pool/DMA.
"""

import functools
import sys

import numpy as np

if "/opt/trn_rl_repo" not in sys.path:
    sys.path.insert(0, "/opt/trn_rl_repo")

HEADS = 4
D = 32              # dim head
C = 256             # channels
TD = 512            # time embedding dim
SCALE = D ** -0.5
N_FULL = 4096       # 64*64 spatial positions
NT = 512            # query (i) tile
JT = 128            # key (j) tile
N_CORES = 8


def _build_program(n_pos=N_FULL):
    import concourse.bass as bass
    import concourse.tile as tile
    from concourse import bacc, mybir
    from concourse.masks import make_identity

    f32 = mybir.dt.float32
    bf16 = mybir.dt.bfloat16
    fp8 = mybir.dt.float8e4
    DR = mybir.MatmulPerfMode.DoubleRow
    AF = mybir.ActivationFunctionType
    ALU = mybir.AluOpType

    nc = bacc.Bacc("TRN2", debug=False)

    xb = nc.dram_tensor("xb", [C, n_pos], f32, kind="ExternalInput").ap()
    te = nc.dram_tensor("te", [TD], f32, kind="ExternalInput").ap()
    w_mlp = nc.dram_tensor("w_mlp", [TD, TD], f32, kind="ExternalInput").ap()
    b_mlp = nc.dram_tensor("b_mlp", [TD], f32, kind="ExternalInput").ap()
    wqkv = nc.dram_tensor("wqkv", [3 * D, C], f32, kind="ExternalInput").ap()
    wo = nc.dram_tensor("wo", [C, D], f32, kind="ExternalInput").ap()
    out = nc.dram_tensor("out", [C, n_pos], f32, kind="ExternalOutput").ap()

    n_itiles = n_pos // NT
    n_jtiles = n_pos // JT
    n_quads = n_jtiles // 4
    XCHUNK = min(1024, n_pos)
    n_xchunks = n_pos // XCHUNK

    with tile.TileContext(nc) as tc:
        with tc.tile_pool(name="const", bufs=1) as const, \
             tc.tile_pool(name="xio", bufs=4) as xio:
            ident = const.tile([128, 128], f32, name="ident")
            make_identity(nc, ident)
            ident_bf = const.tile([128, 128], bf16, name="ident_bf")
            make_identity(nc, ident_bf)

            # FiLM'd x chunks (bf16) + fp32 staging
            x_sb = [[const.tile([128, XCHUNK], bf16, name=f"x_sb{cc}_{k}")
                     for k in range(n_xchunks)] for cc in range(2)]
            q4t = [const.tile([128, NT], bf16, name=f"q4_{it}")
                   for it in range(n_itiles)]
            k4 = const.tile([128, n_pos], bf16, name="k4")
            # V with a ones row appended (row D): the PE transpose emits
            # [128, D+1] V^T tiles with the ones column built in. Row D+1 is
            # 4B-alignment padding, never read.
            v_sb = const.tile([D + 2, n_pos], bf16, name="v_sb")
            VTS = D + 2  # vt block stride
            vt_all = const.tile([128, n_jtiles * VTS], bf16, name="vt_all")

            # FiLM results: sc1 = 1+scale (cols by c-chunk); shift stays in
            # tfull cols 2,3
            tfull = const.tile([128, 4], f32, name="tfull")
            sc1 = const.tile([128, 2], f32, name="sc1")
            # qkv weights, transposed, q/k replicated 4x along the
            # stationary columns for the 4 PE row-groups
            q4T = [const.tile([128, 128], bf16, name=f"q4T{cc}")
                   for cc in range(2)]
            k4T = [const.tile([128, 128], bf16, name=f"k4T{cc}")
                   for cc in range(2)]
            vT_w = [const.tile([128, D], bf16, name=f"vT_w{cc}")
                    for cc in range(2)]
            # proj weights (b_out is added on the host during assembly)
            woT = [const.tile([D, 128], bf16, name=f"woT{hh}")
                   for hh in range(2)]

            # x fp32 staging chunks (DMA'd up front, converted lazily)
            x_stage = [[xio.tile([128, XCHUNK], f32, name=f"x_st_{cc}_{k}",
                                 tag=f"x_st_{cc}_{k}", bufs=1)
                        for k in range(n_xchunks)] for cc in range(2)]

            # -------- DMAs: everything in flight immediately --------
            te_t = const.tile([128, 4], f32, name="te_t")
            nc.sync.dma_start(out=te_t, in_=te.rearrange("(f p) -> p f", p=128))
            # w_mlp first: it gates the FiLM chain (which now also gates the
            # x converts) and shares HBM bandwidth with everything earlier.
            wm_slab = []
            for ot in range(4):
                t_ = xio.tile([128, TD], f32, name=f"wm_slab_{ot}",
                              tag=f"wm_slab_{ot}", bufs=1)
                nc.sync.dma_start(out=t_, in_=w_mlp[ot * 128:(ot + 1) * 128, :])
                wm_slab.append(t_)
            wqkv_nat = const.tile([3 * D, C], f32, name="wqkv_nat")
            nc.sync.dma_start(out=wqkv_nat, in_=wqkv)
            bm_t = const.tile([128, 4], f32, name="bm_t")
            nc.sync.dma_start(out=bm_t, in_=b_mlp.rearrange("(f p) -> p f", p=128))
            n_xsync = min(2, n_xchunks)
            for k in range(n_xsync):
                for cc in range(2):
                    nc.sync.dma_start(
                        out=x_stage[cc][k],
                        in_=xb[128 * cc:128 * (cc + 1),
                               k * XCHUNK:(k + 1) * XCHUNK])
            wo_both = const.tile([128, 2 * D], f32, name="wo_both")
            nc.sync.dma_start(
                out=wo_both.rearrange("p (h d) -> p h d", d=D),
                in_=wo.rearrange("(h p) d -> p h d", p=128))
            for k in range(n_xsync, n_xchunks):
                for cc in range(2):
                    nc.sync.dma_start(
                        out=x_stage[cc][k],
                        in_=xb[128 * cc:128 * (cc + 1),
                               k * XCHUNK:(k + 1) * XCHUNK])

            # PE pipeline warm-up while the DMAs land: the first real
            # matmuls then run at full clock instead of the cold pstate
            with tc.tile_pool(name="warm_ps", bufs=2, space="PSUM") as warm_ps:
                for wi in range(12):
                    wt = warm_ps.tile([128, 128], bf16, tag="w",
                                      name=f"warm_{wi}")
                    nc.tensor.transpose(wt, ident_bf, ident_bf)

            # ones + pad rows of V on the otherwise-idle gpsimd engine
            # (partition slices must start at multiples of 32 -> both rows)
            nc.gpsimd.memset(v_sb[D:D + 2, :], 1.0)

            # ---------------- prologue: FiLM MLP + weight prep --------------
            with tc.tile_pool(name="pro_sb", bufs=3) as pro_sb, \
                 tc.tile_pool(name="pro_ps", bufs=2, space="PSUM") as pro_ps:

                # silu(te) via the EXP table only: s = te / (1 + exp(-te))
                emt = pro_sb.tile([128, 4], f32, tag="emt")
                nc.scalar.activation(emt, te_t, AF.Exp, scale=-1.0)
                nc.vector.tensor_scalar_add(emt, emt, 1.0)
                rec = pro_sb.tile([128, 4], f32, tag="rec")
                nc.vector.reciprocal(rec, emt)
                s_t = const.tile([128, 4], f32, name="s_t")
                nc.vector.tensor_mul(s_t, te_t, rec)
                s_bf = pro_sb.tile([128, 4], bf16, tag="s_bf")
                nc.vector.tensor_copy(s_bf, s_t)

                # W_mlp^T tiles via bf16 PE transpose
                wm_bf = []
                for ot in range(4):
                    wb = pro_sb.tile([128, TD], bf16, tag=f"wm_bf_{ot}",
                                     name=f"wm_bf_{ot}", bufs=1)
                    nc.vector.tensor_copy(wb, wm_slab[ot])
                    wm_bf.append(wb)
                wmT = [[None] * 4 for _ in range(4)]
                for cc4 in range(4):
                    for ot in range(4):
                        ps_t = pro_ps.tile([128, 128], bf16, tag="tp", bufs=3)
                        nc.tensor.transpose(
                            ps_t,
                            wm_bf[ot][:, cc4 * 128:(cc4 + 1) * 128], ident_bf)
                        wmT_t = pro_sb.tile([128, 128], bf16,
                                            tag=f"wmT_{cc4}_{ot}",
                                            name=f"wmT_{cc4}_{ot}", bufs=1)
                        nc.vector.tensor_copy(wmT_t, ps_t)
                        wmT[cc4][ot] = wmT_t

                # t = W_mlp @ silu(te) + b_mlp
                for ot in range(4):
                    t_ps = pro_ps.tile([128, 1], f32, tag="t_ps")
                    for cc4 in range(4):
                        nc.tensor.matmul(
                            t_ps, wmT[cc4][ot], s_bf[:, cc4:cc4 + 1],
                            start=(cc4 == 0), stop=(cc4 == 3),
                        )
                    nc.vector.tensor_add(
                        tfull[:, ot:ot + 1], t_ps, bm_t[:, ot:ot + 1]
                    )
                # 1+scale for the two c-chunks; shift = tfull cols 2,3
                nc.vector.tensor_scalar_add(sc1, tfull[:, 0:2], 1.0)
                shf = tfull[:, 2:4]

                # W_qkv head slices -> bf16 -> transpose; q/k 16-col halves
                # duplicated across the stationary tiles, v plain
                wqkv_bf = pro_sb.tile([3 * D, C], bf16, tag="wqkv_bf")
                nc.vector.tensor_copy(wqkv_bf, wqkv_nat)
                w_bf = {"q": wqkv_bf[0:D, :], "k": wqkv_bf[D:2 * D, :],
                        "v": wqkv_bf[2 * D:3 * D, :]}
                w_base = {"q": 0, "k": D, "v": 2 * D}
                for name, Wdst in (("k", k4T), ("q", q4T), ("v", None)):
                    b0 = w_base[name]
                    for cc in range(2):
                        ps_t = pro_ps.tile([128, D], bf16, tag="tp", bufs=3)
                        nc.tensor.transpose(
                            ps_t, w_bf[name][:, 128 * cc:128 * (cc + 1)],
                            ident_bf[b0:b0 + D, b0:b0 + D],
                        )
                        if name == "v":
                            nc.vector.tensor_copy(vT_w[cc], ps_t)
                        else:
                            w = Wdst[cc]
                            nc.vector.tensor_copy(w[:, 0:D], ps_t)
                            nc.vector.tensor_copy(w[:, D:2 * D], w[:, 0:D])
                            nc.vector.tensor_copy(
                                w[:, 2 * D:4 * D], w[:, 0:2 * D])

                # w_out^T halves (b_out is added host-side)
                wo_bf = pro_sb.tile([128, 2 * D], bf16, tag="wo_bf")
                nc.vector.tensor_copy(wo_bf, wo_both)
                for hh in range(2):
                    ps_t = pro_ps.tile([D, 128], bf16, tag="tp", bufs=3)
                    nc.tensor.transpose(ps_t,
                                        wo_bf[:, D * hh:D * (hh + 1)], ident_bf)
                    nc.vector.tensor_copy(woT[hh], ps_t)

            # ---------------- attention + output projection ----------------
            with tc.tile_pool(name="sc_ps", bufs=3, space="PSUM") as sc_ps, \
                 tc.tile_pool(name="u_ps", bufs=2, space="PSUM") as u_ps, \
                 tc.tile_pool(name="pt_sb", bufs=6) as pt_sb, \
                 tc.tile_pool(name="o_sb", bufs=3) as o_sb:

                converted = [False] * n_xchunks

                def film_act(k):
                    # FiLM'd conversion on the scalar engine:
                    # x~ = Identity(x * (1+scale) + shift), fp32 -> bf16
                    for cc in range(2):
                        nc.scalar.activation(
                            x_sb[cc][k], x_stage[cc][k], AF.Identity,
                            bias=shf[:, cc:cc + 1], scale=sc1[:, cc:cc + 1])

                def ensure_chunk(k):
                    if converted[k]:
                        return
                    converted[k] = True
                    # FiLM'd conversion on DVE
                    for cc in range(2):
                        nc.vector.tensor_scalar(
                            x_sb[cc][k], x_stage[cc][k],
                            sc1[:, cc:cc + 1], shf[:, cc:cc + 1],
                            ALU.mult, ALU.add)

                # first chunks on the (pre-loop idle) scalar engine
                n_pro = n_xsync
                for k in range(n_pro):
                    converted[k] = True
                    film_act(k)

                def emit_qkv(nt):
                    # qkv projection for one n-tile; k first (it gates the
                    # i-tile-0 score matmuls), then q and v.
                    ensure_chunk(nt * NT // XCHUNK)
                    ensure_chunk(((nt + 1) * NT - 1) // XCHUNK)
                    sl = slice(nt * NT, (nt + 1) * NT)
                    kx = nt * NT // XCHUNK
                    lo = nt * NT - kx * XCHUNK
                    xch = [x_sb[cc][kx][:, lo:lo + NT] for cc in range(2)]
                    ps_k = sc_ps.tile([128, NT], f32, tag="sc",
                                      name=f"psk_{nt}")
                    for cc in range(2):
                        nc.tensor.matmul(ps_k, k4T[cc], xch[cc],
                                         start=(cc == 0), stop=(cc == 1))
                    nc.vector.tensor_copy(k4[:, sl], ps_k)
                    ps_q = sc_ps.tile([128, NT], f32, tag="sc",
                                      name=f"psq_{nt}")
                    for cc in range(2):
                        nc.tensor.matmul(ps_q, q4T[cc], xch[cc],
                                         start=(cc == 0), stop=(cc == 1))
                    nc.vector.tensor_copy(q4t[nt], ps_q)
                    ps_v = sc_ps.tile([D, NT], f32, tag="sc",
                                      name=f"psv_{nt}")
                    for cc in range(2):
                        nc.tensor.matmul(ps_v, vT_w[cc], xch[cc],
                                         start=(cc == 0), stop=(cc == 1))
                    nc.vector.tensor_copy(v_sb[0:D, sl], ps_v)

                def emit_vt(nt):
                    # V^T for the 4 j-tiles of this n-tile; AFTER the score
                    # matmuls of the group so the in-order PE queue never
                    # delays scores on v.
                    ps_vt = sc_ps.tile([128, 4 * VTS], bf16, tag="sc",
                                       name=f"psvt_{nt}")
                    for jj in range(4):
                        j = 4 * nt + jj
                        nc.tensor.transpose(
                            ps_vt[:, jj * VTS:(jj + 1) * VTS],
                            v_sb[:, j * JT:(j + 1) * JT],
                            ident_bf[0:VTS, 0:VTS])
                    nc.vector.tensor_copy(
                        vt_all[:, 4 * nt * VTS:(4 * nt + 4) * VTS],
                        ps_vt)

                def score_mm(Sx, off, j, r, qt):
                    nc.tensor.matmul(
                        Sx[:, off:off + NT],
                        k4[D * r:D * (r + 1), j * JT:(j + 1) * JT],
                        qt[D * r:D * (r + 1), :],
                        start=True, stop=True, tile_position=(32 * r, 0),
                    )

                def emit_scores0(g):
                    # i-tile-0 scores with a consumerless dummy allocation
                    # between S1 and S2: the pool rotation then maps every
                    # reuse onto an early DVE copy instead of an exp
                    # completion, decoupling the qkv build from the exp
                    # cadence.
                    S1 = sc_ps.tile([128, 2 * NT], f32, tag="sc",
                                    name=f"S1_0_{g}")
                    for r, off in ((0, 0), (1, NT)):
                        score_mm(S1, off, 4 * g + r, r, q4t[0])
                    sc_ps.tile([128, 8], f32, tag="sc", name=f"dummy_{g}")
                    S2 = sc_ps.tile([128, 2 * NT], f32, tag="sc",
                                    name=f"S2_0_{g}")
                    for r, off in ((2, 0), (3, NT)):
                        score_mm(S2, off, 4 * g + r, r, q4t[0])
                    PT1 = pt_sb.tile([128, 2 * NT], bf16, tag="pt",
                                     name=f"PT1_0_{g}")
                    nc.scalar.activation(PT1, S1, AF.Exp, scale=SCALE)
                    PT2 = pt_sb.tile([128, 2 * NT], bf16, tag="pt",
                                     name=f"PT2_0_{g}")
                    nc.scalar.activation(PT2, S2, AF.Exp, scale=SCALE)
                    return PT1, PT2

                def emit_scores(it, g):
                    S1 = sc_ps.tile([128, 2 * NT], f32, tag="sc",
                                    name=f"S1_{it}_{g}")
                    S2 = sc_ps.tile([128, 2 * NT], f32, tag="sc",
                                    name=f"S2_{it}_{g}")
                    for r, (Sx, off) in enumerate(
                            ((S1, 0), (S1, NT), (S2, 0), (S2, NT))):
                        score_mm(Sx, off, 4 * g + r, r, q4t[it])
                    PT1 = pt_sb.tile([128, 2 * NT], bf16, tag="pt",
                                     name=f"PT1_{it}_{g}")
                    nc.scalar.activation(PT1, S1, AF.Exp, scale=SCALE)
                    PT2 = pt_sb.tile([128, 2 * NT], bf16, tag="pt",
                                     name=f"PT2_{it}_{g}")
                    nc.scalar.activation(PT2, S2, AF.Exp, scale=SCALE)
                    return PT1, PT2

                def emit_u(U, g, PT1, PT2):
                    st, sp = (g == 0), (g == n_quads - 1)
                    for idx, (PTx, off) in enumerate(
                            ((PT1, 0), (PT1, NT), (PT2, 0), (PT2, NT))):
                        j = 4 * g + idx
                        lo = 0 if idx % 2 == 0 else 64
                        nc.tensor.matmul(
                            U[lo:lo + D + 1, :],
                            vt_all[:, j * VTS:j * VTS + D + 1],
                            PTx[:, off:off + NT],
                            start=(st and idx < 2), stop=(sp and idx >= 2),
                            tile_position=(0, lo),
                            skip_group_check=True,
                        )

                def emit_usum(it, U):
                    # combine the two column-group halves (one PSUM operand
                    # per DVE op); bf16 usum for the proj matmul, f32
                    # denominator row for the reciprocal
                    usum_b = o_sb.tile([D + 1, NT], f32, tag="usum_b",
                                       name=f"usum_b_{it}")
                    nc.vector.tensor_copy(usum_b, U[64:64 + D + 1, :])
                    den = o_sb.tile([1, NT], f32, tag="den", name=f"den_{it}")
                    nc.vector.tensor_add(den, U[D:D + 1, :],
                                         usum_b[D:D + 1, :])
                    usum = o_sb.tile([D, NT], bf16, tag="usum",
                                     name=f"usum_{it}")
                    nc.vector.tensor_add(usum, U[0:D, :], usum_b[0:D, :])
                    return usum, den

                def emit_recip_dve(it, den):
                    # [1, NT] reciprocal: ~2.6us of idle DVE, off the PE path
                    rcp = o_sb.tile([1, NT], f32, tag="rcp", name=f"rcp_{it}")
                    nc.vector.reciprocal(rcp, den)
                    rb = o_sb.tile([128, NT], f32, tag="rb", name=f"rb_{it}")
                    nc.gpsimd.partition_broadcast(rb, rcp)
                    return rb

                def emit_recip_tail(it, den, width, off, sfx):
                    # transposed reciprocal for the tail chunks: PE transpose
                    # -> [128, nq] reciprocal -> transpose back -> broadcast
                    nq = width // 128
                    denT_ps = sc_ps.tile([128, nq], f32, tag="sc",
                                         name=f"denT_{it}{sfx}")
                    for q in range(nq):
                        nc.tensor.transpose(
                            denT_ps[:, q:q + 1],
                            den[:, off + 128 * q:off + 128 * (q + 1)],
                            ident[0:1, 0:1])
                    rcpT = o_sb.tile([128, nq], f32, tag="rcpT",
                                     name=f"rcpT_{it}{sfx}")
                    nc.vector.reciprocal(rcpT, denT_ps)
                    row_ps = sc_ps.tile([1, width], f32, tag="sc",
                                        name=f"rcp_row_{it}{sfx}")
                    for q in range(nq):
                        nc.tensor.transpose(
                            row_ps[:, 128 * q:128 * (q + 1)],
                            rcpT[:, q:q + 1], ident)
                    rcp_row = o_sb.tile([1, width], f32, tag="rcp",
                                        name=f"rcp{it}{sfx}")
                    nc.vector.tensor_copy(rcp_row, row_ps)
                    rb = o_sb.tile([128, width], f32, tag="rb",
                                   name=f"rb_{it}{sfx}")
                    nc.gpsimd.partition_broadcast(rb, rcp_row)
                    return rb

                def emit_proj_mm(pit, usum, width=NT, off=0, sfx=""):
                    ps_os = []
                    for hh in range(2):
                        ps_o = sc_ps.tile([128, width], f32, tag="sc",
                                          name=f"ps_o_{pit}_{hh}{sfx}")
                        nc.tensor.matmul(ps_o, woT[hh],
                                         usum[:, off:off + width],
                                         start=True, stop=True)
                        ps_os.append(ps_o)
                    return ps_os

                def emit_proj_fin(pit, ps_os, rb, width=NT, off=0, sfx=""):
                    psl = slice(pit * NT + off, pit * NT + off + width)
                    for hh in range(2):
                        o_out = o_sb.tile([128, width], f32, tag="o_out",
                                          name=f"o_out_{pit}_{hh}{sfx}")
                        nc.vector.tensor_mul(o_out, ps_os[hh], rb)
                        nc.sync.dma_start(
                            out=out[128 * hh:128 * (hh + 1), psl], in_=o_out
                        )

                # i-tile 0 fused with the qkv/V^T build: quad g only needs
                # k/v/VT of n-tile g (and q of n-tile 0). Software-pipelined:
                # the last U matmuls of i-tile it-1 and its norm/proj are
                # emitted inside i-tile it's block.
                assert n_quads == n_itiles
                U0 = u_ps.tile([97, NT], f32, tag="u", name="U_0")
                emit_qkv(0)
                prev = emit_scores0(0)
                emit_vt(0)
                for g in range(1, n_quads):
                    kx = n_xsync + (g - 2) // 2
                    if g >= 2 and g % 2 == 0 and kx < n_xchunks:
                        # late x chunks: FiLM'd in i-tile 0's ACT idle slots
                        converted[kx] = True
                        film_act(kx)
                    emit_qkv(g)
                    cur = emit_scores0(g)
                    emit_vt(g)
                    emit_u(U0, g - 1, *prev)
                    prev = cur
                carry = (U0, 0, prev)

                for it in range(1, n_itiles):
                    U = u_ps.tile([97, NT], f32, tag="u", name=f"U_{it}")
                    prev = emit_scores(it, 0)
                    Uc, itc, sc_prev = carry
                    emit_u(Uc, n_quads - 1, *sc_prev)
                    usum_c, den_c = emit_usum(itc, Uc)
                    rb_c = None
                    for g in range(1, n_quads):
                        cur = emit_scores(it, g)
                        emit_u(U, g - 1, *prev)
                        prev = cur
                        if g == 3:
                            rb_c = emit_recip_dve(itc, den_c)
                    if rb_c is None:
                        rb_c = emit_recip_dve(itc, den_c)
                    # proj of it-1 at the END of the block: its PSUM slot's
                    # consumer (the normalize multiply) then runs right away,
                    # so the rotation never blocks the next i-tile's scores.
                    pmm = emit_proj_mm(itc, usum_c)
                    emit_proj_fin(itc, pmm, rb_c)
                    carry = (U, it, prev)

                # tail: last i-tile's normalization in 128-col chunks,
                # pipelined across PE/DVE/Pool/DMA
                Uc, itc, sc_prev = carry
                emit_u(Uc, n_quads - 1, *sc_prev)
                usum_t, den_t = emit_usum(itc, Uc)
                CH = 128
                for h2 in range(NT // CH):
                    off = h2 * CH
                    rb_h = emit_recip_tail(itc, den_t, CH, off, f"_t{h2}")
                    pmm = emit_proj_mm(itc, usum_t, width=CH, off=off,
                                       sfx=f"_t{h2}")
                    emit_proj_fin(itc, pmm, rb_h, width=CH, off=off,
                                  sfx=f"_t{h2}")
    nc.compile()
    return nc


@functools.lru_cache(maxsize=2)
def _get_nc(n_pos=N_FULL):
    return _build_program(n_pos)


def _make_in_maps(x, time_emb, w_mlp, b_mlp, w_qkv, w_out, b_out, n_pos=N_FULL):
    x = np.ascontiguousarray(np.asarray(x, dtype=np.float32))
    time_emb = np.ascontiguousarray(np.asarray(time_emb, dtype=np.float32))
    w_mlp = np.ascontiguousarray(np.asarray(w_mlp, dtype=np.float32))
    b_mlp = np.ascontiguousarray(np.asarray(b_mlp, dtype=np.float32))
    w_qkv = np.ascontiguousarray(np.asarray(w_qkv, dtype=np.float32))
    w_out = np.ascontiguousarray(np.asarray(w_out, dtype=np.float32))
    b_out = np.ascontiguousarray(np.asarray(b_out, dtype=np.float32))

    b = x.shape[0]
    hid = HEADS * D
    in_maps = []
    for core in range(N_CORES):
        bb, hh = core // HEADS, core % HEADS
        in_maps.append({
            "xb": np.ascontiguousarray(
                x[bb].reshape(C, -1)[:, :n_pos]),
            "te": time_emb[bb],
            "w_mlp": w_mlp,
            "b_mlp": b_mlp,
            "wqkv": np.ascontiguousarray(np.concatenate([
                w_qkv[D * hh:D * (hh + 1), :],
                w_qkv[hid + D * hh:hid + D * (hh + 1), :],
                w_qkv[2 * hid + D * hh:2 * hid + D * (hh + 1), :]], axis=0)),
            "wo": np.ascontiguousarray(w_out[:, D * hh:D * (hh + 1)]),
        })
    return in_maps


def _install_ntff_hook():
    """Register the axon NTFF profile hook (the agent image's antenv lacks
    axon_hooks; replicate trn_boot's ctypes shim so trace=True works)."""
    import types
    import contextlib
    import ctypes

    try:
        from antenv.axon_hooks import get_axon_ntff_profile_hook  # noqa: F401
        return
    except ImportError:
        pass
    so_path = "/opt/axon/libaxon_pjrt.so"
    try:
        lib = ctypes.CDLL(so_path)
    except OSError:
        return
    if not hasattr(lib, "axon_start_nrt_profile"):
        return
    lib.axon_start_nrt_profile.argtypes = [
        ctypes.POINTER(ctypes.c_int64), ctypes.c_size_t]
    lib.axon_start_nrt_profile.restype = ctypes.c_int64
    lib.axon_stop_nrt_profile.argtypes = [ctypes.c_char_p]
    lib.axon_stop_nrt_profile.restype = ctypes.c_int64

    @contextlib.contextmanager
    def _hook(output_dir, device_ids):
        import jax
        jax.devices()
        if device_ids:
            ids = (ctypes.c_int64 * len(device_ids))(*device_ids)
            rc = lib.axon_start_nrt_profile(ids, len(device_ids))
        else:
            rc = lib.axon_start_nrt_profile(None, 0)
        if rc != 0:
            raise RuntimeError(f"axon_start_nrt_profile rc={rc}")
        try:
            yield
        finally:
            n = lib.axon_stop_nrt_profile(str(output_dir).encode())
            print(f"profile: {n} file(s) written to {output_dir}",
                  file=sys.stderr)

    import antenv
    mod = types.ModuleType("antenv.axon_hooks")
    mod.get_axon_ntff_profile_hook = lambda: _hook
    mod.set_axon_ntff_profile_hook = lambda h: None
    sys.modules["antenv.axon_hooks"] = mod
    antenv.axon_hooks = mod


def _run(inputs, trace=False, n_pos=N_FULL):
    from concourse.bass_utils import run_bass_kernel_spmd

    if trace:
        _install_ntff_hook()
    nc = _get_nc(n_pos)
    in_maps = _make_in_maps(**inputs, n_pos=n_pos)
    res = run_bass_kernel_spmd(
        nc, in_maps, core_ids=list(range(N_CORES)), trace=trace
    )
    return res


def _assemble(results, x_shape, b_out):
    b, c, h, w = x_shape
    out = np.zeros((b, c, h * w), dtype=np.float32)
    for core in range(N_CORES):
        bb = core // HEADS
        out[bb] += results[core]["out"]
    out += np.asarray(b_out, dtype=np.float32)[None, :, None]
    return out.reshape(b, c, h, w)


def kernel(x, time_emb, w_mlp, b_mlp, w_qkv, w_out, b_out):
    res = _run(dict(
        x=x, time_emb=time_emb, w_mlp=w_mlp, b_mlp=b_mlp,
        w_qkv=w_qkv, w_out=w_out, b_out=b_out,
    ))
    return _assemble(res.results, np.asarray(x).shape, b_out)
